# revision 13
# baseline (speedup 1.0000x reference)
"""Multi-head causal attention (B=2, T=2048, C=1024, H=16, D=64) on 8 trn2 cores.

Sharding: core c -> batch b = c//4, head group g = c%4 (4 heads each).
Megatron-style: QKV column-parallel, proj row-parallel; partial outputs are
summed on the host (bk is softmax-invariant and dropped; bv/bp fold into a
host-side constant).

Device kernel (per core), matmuls in fp32r:
  A = x[b].T                       [1024, 2048]  (host-transposed)
  Q^T (+bq) / K^T = W.T @ A        [256, 2048]   channels on partitions
  V = A.T @ Wv_loc.T               [2048, 4*(64+1)]  natural layout, a ones
                                   column per head for softmax denominators
  per head h, 1024-wide q-window qjj, 128-key chunk kc:
     S^T[k,q] = K_h^T.T @ Q_h^T    PSUM [128, <=1024]
     P^T = exp(0.125*S^T)          single ACT op per window (ACT runs Exp only)
     diag chunks masked on GpSimd via affine_select
     PV~[65,512] += V~_h[kc].T @ P^T   row 64 accumulates the denominator l
     out^T = PV[0:64] * bcast(1/l)     (approx recip + gpsimd partition_broadcast)
  Y = attn-out^T.T @ Wp_loc.T      [2048, 1024]  partial, summed on host
"""

import sys

sys.path.insert(0, "/opt/trn_rl_repo")

import numpy as np
import ml_dtypes

NP_DT = ml_dtypes.bfloat16

import concourse.bass as bass  # noqa: F401
import concourse.mybir as mybir
import concourse.tile as tile
from concourse import bacc
from concourse.bass_utils import run_bass_kernel_spmd

N_CORES = 8
B, T, C = 2, 2048, 1024
H, D = 16, 64
H_LOC = 4              # heads per core
OL = H_LOC * D         # local channels = 256
CQ = 512               # PSUM-bank q chunk
CW = 1024              # exp window (2 PSUM banks)
CK = 128               # k chunk (partition dim)
NW = T // CW           # 2
NT = T // 128          # 16
KC = C // 128          # 8 contraction chunks for QKV

f32 = mybir.dt.float32
f32r = mybir.dt.float32r
bf16 = mybir.dt.bfloat16
DT = bf16  # matmul operand dtype

_COMPILED = None


def _build():
    nc = bacc.Bacc("TRN2", debug=False, num_devices=N_CORES)

    A = nc.dram_tensor("A", [C, T], DT, kind="ExternalInput").ap()
    Wqkv = nc.dram_tensor("Wqkv", [C, 3 * OL], DT, kind="ExternalInput").ap()
    WpT = nc.dram_tensor("WpT", [OL, C], DT, kind="ExternalInput").ap()
    BQ = nc.dram_tensor("BQ", [OL, 1], f32, kind="ExternalInput").ap()
    Y = nc.dram_tensor("Y", [T, C], f32, kind="ExternalOutput").ap()

    Exp = mybir.ActivationFunctionType.Exp

    with tile.TileContext(nc) as tc:
        with tc.tile_pool(name="sbuf", bufs=1) as pool, \
             tc.tile_pool(name="work", bufs=1) as wpool, \
             tc.tile_pool(name="psum", bufs=1, space="PSUM") as psum:

            # ---- resident inputs (split loads so compute starts early) ----
            a_t, w_t = [], []
            for kc in range(KC):
                at = pool.tile([128, T], DT, tag=f"A{kc}", name=f"a{kc}")
                a_t.append(at)
                wt = pool.tile([128, 3 * OL], DT, tag=f"W{kc}", name=f"w{kc}")
                w_t.append(wt)
                nc.sync.dma_start(wt[:], Wqkv[kc * 128:(kc + 1) * 128, :])
                for piece in range(4):
                    nc.sync.dma_start(
                        at[:, piece * CQ:(piece + 1) * CQ],
                        A[kc * 128:(kc + 1) * 128, piece * CQ:(piece + 1) * CQ])
            wp_t = []
            for kc in range(2):
                wp = pool.tile([128, C], DT, tag=f"WP{kc}", name=f"wp{kc}")
                nc.sync.dma_start(wp[:], WpT[kc * 128:(kc + 1) * 128, :])
                wp_t.append(wp)
            bq_t = []
            for m in range(2):
                bq = pool.tile([128, 1], f32, tag=f"BQ{m}", name=f"bq{m}")
                nc.sync.dma_start(bq[:], BQ[m * 128:(m + 1) * 128, :])
                bq_t.append(bq)
            col1 = pool.tile([128, 1], f32, tag="col1")
            nc.vector.memset(col1[:], 1.0)

            # ---- persistent intermediates ----
            qt_sb = [pool.tile([128, T], DT, tag=f"QT{i}", name=f"qt{i}")
                     for i in range(2)]
            kt_sb = [pool.tile([128, T], DT, tag=f"KT{i}", name=f"kt{i}")
                     for i in range(2)]
            v_sb = [pool.tile([128, H_LOC * (D + 1)], DT, tag=f"V{i}",
                              name=f"v{i}") for i in range(NT)]
            ao_sb = [pool.tile([128, T], DT, tag=f"AO{i}", name=f"ao{i}")
                     for i in range(2)]

            # ---- phase 1a: Q^T, K^T (evict on DVE; ACT is Exp-only) ----
            for m in range(4):
                for n in range(T // CQ):
                    ps = psum.tile([128, CW], f32, tag="mm", bufs=3, name="ps")[:, 0:CQ]
                    for kc in range(KC):
                        nc.tensor.matmul(
                            ps[:],
                            w_t[kc][:, m * 128:(m + 1) * 128],
                            a_t[kc][:, n * CQ:(n + 1) * CQ],
                            start=(kc == 0), stop=(kc == KC - 1))
                    if m < 2:
                        nc.vector.tensor_scalar_add(
                            qt_sb[m][:, n * CQ:(n + 1) * CQ], ps[:],
                            bq_t[m][:, 0:1])
                    else:
                        nc.vector.tensor_copy(
                            kt_sb[m - 2][:, n * CQ:(n + 1) * CQ], ps[:])

            # ---- phase 1b: V natural layout ----
            for tt in range(NT):
                ps = psum.tile([128, CW], f32, tag="mm", bufs=3, name="psv")[:, 0:OL]
                for kc in range(KC):
                    nc.tensor.matmul(
                        ps[:],
                        a_t[kc][:, tt * 128:(tt + 1) * 128],
                        w_t[kc][:, 2 * OL:3 * OL],
                        start=(kc == 0), stop=(kc == KC - 1))
                for h in range(H_LOC):
                    nc.vector.tensor_copy(
                        v_sb[tt][:, h * (D + 1):h * (D + 1) + D],
                        ps[:, h * D:(h + 1) * D])
                    nc.vector.tensor_copy(
                        v_sb[tt][:, h * (D + 1) + D:(h + 1) * (D + 1)],
                        col1[:])

            # ---- phase 2+3: causal flash attention, proj interleaved ----
            for qjj in range(NW):              # 1024-wide q window
                for h in range(H_LOC):
                    ht, hp = h // 2, (h % 2) * 64
                    n_kc = (qjj + 1) * (CW // CK)
                    pv0 = psum.tile([D + 1, CQ], f32, tag="pv", bufs=2)
                    pv1 = psum.tile([D + 1, CQ], f32, tag="pv", bufs=2)
                    q0 = qjj * CW              # window start
                    # per-half contraction depths (causal)
                    nk0 = (qjj * 2 + 1) * 4    # half0 needs kc < nk0
                    nk1 = (qjj * 2 + 2) * 4
                    for kc in range(n_kc):
                        # which q-halves need this key chunk
                        use0 = kc < nk0
                        w = CW if use0 else CQ
                        qoff = q0 if use0 else q0 + CQ
                        sp = psum.tile([128, CW], f32, tag="mm", bufs=3)
                        for half in range(2 if use0 else 1):
                            nc.tensor.matmul(
                                sp[:, half * CQ:(half + 1) * CQ],
                                kt_sb[ht][hp:hp + D, kc * CK:(kc + 1) * CK],
                                qt_sb[ht][hp:hp + D,
                                          qoff + half * CQ:qoff + (half + 1) * CQ],
                                start=True, stop=True)
                        pt = wpool.tile([128, CW], DT, tag="pT", bufs=6)
                        nc.scalar.activation(pt[:, 0:w], sp[:, 0:w], Exp,
                                             scale=1.0 / 8.0)
                        # causal mask: at most one diagonal 512-half per kc;
                        # it always sits at pt[:, 0:CQ] (half1-only case is
                        # stored there too). base = qstart - kc*CK.
                        if use0 and kc >= qjj * 8:
                            dbase = q0 - kc * CK          # half0 diagonal
                        elif not use0:
                            dbase = (q0 + CQ) - kc * CK   # half1 diagonal
                        else:
                            dbase = None
                        if dbase is not None:
                            nc.gpsimd.affine_select(
                                out=pt[:, 0:CQ], in_=pt[:, 0:CQ],
                                compare_op=mybir.AluOpType.is_ge,
                                fill=0.0, base=dbase,
                                pattern=[[1, CQ]], channel_multiplier=-1)
                        # PV per 512 half
                        if use0:
                            nc.tensor.matmul(
                                pv0[:],
                                v_sb[kc][:, h * (D + 1):(h + 1) * (D + 1)],
                                pt[:, 0:CQ],
                                start=(kc == 0), stop=(kc == nk0 - 1))
                            nc.tensor.matmul(
                                pv1[:],
                                v_sb[kc][:, h * (D + 1):(h + 1) * (D + 1)],
                                pt[:, CQ:CW],
                                start=(kc == 0), stop=(kc == nk1 - 1))
                        else:
                            nc.tensor.matmul(
                                pv1[:],
                                v_sb[kc][:, h * (D + 1):(h + 1) * (D + 1)],
                                pt[:, 0:CQ],
                                start=(kc == 0), stop=(kc == nk1 - 1))
                    for half, pv in ((0, pv0), (1, pv1)):
                        # approx_fast cannot read PSUM on HW; bounce via SBUF
                        ls = wpool.tile([1, CQ], f32, tag="ls", bufs=2)
                        nc.vector.tensor_copy(ls[:], pv[D:D + 1, :])
                        r = wpool.tile([1, CQ], f32, tag="r", bufs=2)
                        with nc.allow_low_precision(reason="softmax denom"):
                            nc.vector.reciprocal_approx_fast(r[:], ls[:])
                        rbs = wpool.tile([D, CQ], f32, tag="rbs", bufs=2)
                        nc.gpsimd.partition_broadcast(rbs[:], r[:])
                        qs = q0 + half * CQ
                        nc.vector.tensor_mul(
                            ao_sb[ht][hp:hp + D, qs:qs + CQ],
                            pv[0:D, :], rbs[:])

                # proj + store for this window's token tiles (overlaps the
                # next window's attention)
                for tt in range(qjj * (CW // 128), (qjj + 1) * (CW // 128)):
                    for n in range(2):
                        ps = psum.tile([128, CW], f32, tag="mm", bufs=3,
                                       name="ps")[:, 0:CQ]
                        for kc in range(2):
                            nc.tensor.matmul(
                                ps[:],
                                ao_sb[kc][:, tt * 128:(tt + 1) * 128],
                                wp_t[kc][:, n * CQ:(n + 1) * CQ],
                                start=(kc == 0), stop=(kc == 1))
                        yt = wpool.tile([128, CQ], f32, tag="y", bufs=3)
                        nc.vector.tensor_copy(yt[:], ps[:])
                        nc.sync.dma_start(
                            Y[tt * 128:(tt + 1) * 128, n * CQ:(n + 1) * CQ],
                            yt[:])

    nc.compile()
    return nc


def _get_compiled():
    global _COMPILED
    if _COMPILED is None:
        _COMPILED = _build()
    return _COMPILED


def kernel(x, Wq, bq, Wk, bk, Wv, bv, Wp, bp):
    x = np.asarray(x, dtype=np.float32)
    Wq = np.asarray(Wq, dtype=np.float32)
    bq = np.asarray(bq, dtype=np.float32)
    Wk = np.asarray(Wk, dtype=np.float32)
    Wv = np.asarray(Wv, dtype=np.float32)
    Wp = np.asarray(Wp, dtype=np.float32)
    bv = np.asarray(bv, dtype=np.float32)
    bp = np.asarray(bp, dtype=np.float32)

    nc = _get_compiled()

    in_maps = []
    for c in range(N_CORES):
        b, g = divmod(c, 4)
        sl = slice(g * OL, (g + 1) * OL)
        in_maps.append({
            "A": np.ascontiguousarray(x[b].T).astype(NP_DT),
            "Wqkv": np.concatenate([Wq[sl].T, Wk[sl].T, Wv[sl].T], axis=1).astype(NP_DT),
            "WpT": np.ascontiguousarray(Wp[:, sl].T).astype(NP_DT),
            "BQ": bq[sl].reshape(OL, 1),
        })

    res = run_bass_kernel_spmd(nc, in_maps, core_ids=list(range(N_CORES)))

    extra = bv @ Wp.T + bp  # bv/bp fold out of the device kernel
    out = np.empty((B, T, C), dtype=np.float32)
    for b in range(B):
        acc = res.results[4 * b]["Y"].astype(np.float32)
        for g in range(1, 4):
            acc = acc + res.results[4 * b + g]["Y"]
        out[b] = acc + extra
    return out


# revision 15
# speedup vs baseline: 1.0771x; 1.0771x over previous
"""Multi-head causal attention (B=2, T=2048, C=1024, H=16, D=64) on 8 trn2 cores.

Sharding: core c -> batch b = c//4, head group g = c%4 (4 heads each).
Megatron-style: QKV column-parallel, proj row-parallel; partial outputs are
summed on the host (bk is softmax-invariant and dropped; bv/bp fold into a
host-side constant).

Device kernel (per core), matmuls in fp32r:
  A = x[b].T                       [1024, 2048]  (host-transposed)
  Q^T (+bq) / K^T = W.T @ A        [256, 2048]   channels on partitions
  V = A.T @ Wv_loc.T               [2048, 4*(64+1)]  natural layout, a ones
                                   column per head for softmax denominators
  per head h, 1024-wide q-window qjj, 128-key chunk kc:
     S^T[k,q] = K_h^T.T @ Q_h^T    PSUM [128, <=1024]
     P^T = exp(0.125*S^T)          single ACT op per window (ACT runs Exp only)
     diag chunks masked on GpSimd via affine_select
     PV~[65,512] += V~_h[kc].T @ P^T   row 64 accumulates the denominator l
     out^T = PV[0:64] * bcast(1/l)     (approx recip + gpsimd partition_broadcast)
  Y = attn-out^T.T @ Wp_loc.T      [2048, 1024]  partial, summed on host
"""

import sys

sys.path.insert(0, "/opt/trn_rl_repo")

import numpy as np
import ml_dtypes

NP_DT = ml_dtypes.bfloat16

import concourse.bass as bass  # noqa: F401
import concourse.mybir as mybir
import concourse.tile as tile
from concourse import bacc
from concourse.bass_utils import run_bass_kernel_spmd

N_CORES = 8
B, T, C = 2, 2048, 1024
H, D = 16, 64
H_LOC = 4              # heads per core
OL = H_LOC * D         # local channels = 256
CQ = 512               # PSUM-bank q chunk
CW = 1024              # exp window (2 PSUM banks)
CK = 128               # k chunk (partition dim)
NW = T // CW           # 2
NT = T // 128          # 16
KC = C // 128          # 8 contraction chunks for QKV

f32 = mybir.dt.float32
f32r = mybir.dt.float32r
bf16 = mybir.dt.bfloat16
DT = bf16  # matmul operand dtype

_COMPILED = None


def _build():
    nc = bacc.Bacc("TRN2", debug=False, num_devices=N_CORES)

    A = nc.dram_tensor("A", [C, T], DT, kind="ExternalInput").ap()
    Wqkv = nc.dram_tensor("Wqkv", [C, 3 * OL], DT, kind="ExternalInput").ap()
    WpT = nc.dram_tensor("WpT", [OL, C], DT, kind="ExternalInput").ap()
    BQ = nc.dram_tensor("BQ", [OL, 1], f32, kind="ExternalInput").ap()
    Y = nc.dram_tensor("Y", [T, C], f32, kind="ExternalOutput").ap()

    Exp = mybir.ActivationFunctionType.Exp

    with tile.TileContext(nc) as tc:
        with tc.tile_pool(name="sbuf", bufs=1) as pool, \
             tc.tile_pool(name="work", bufs=1) as wpool, \
             tc.tile_pool(name="psum", bufs=1, space="PSUM") as psum:

            # ---- resident inputs (split loads so compute starts early) ----
            a_t, w_t = [], []
            for kc in range(KC):
                at = pool.tile([128, T], DT, tag=f"A{kc}", name=f"a{kc}")
                a_t.append(at)
                wt = pool.tile([128, 3 * OL], DT, tag=f"W{kc}", name=f"w{kc}")
                w_t.append(wt)
                nc.sync.dma_start(wt[:], Wqkv[kc * 128:(kc + 1) * 128, :])
                for piece in range(4):
                    nc.sync.dma_start(
                        at[:, piece * CQ:(piece + 1) * CQ],
                        A[kc * 128:(kc + 1) * 128, piece * CQ:(piece + 1) * CQ])
            wp_t = []
            for kc in range(2):
                wp = pool.tile([128, C], DT, tag=f"WP{kc}", name=f"wp{kc}")
                nc.sync.dma_start(wp[:], WpT[kc * 128:(kc + 1) * 128, :])
                wp_t.append(wp)
            bq_t = []
            for m in range(2):
                bq = pool.tile([128, 1], f32, tag=f"BQ{m}", name=f"bq{m}")
                nc.sync.dma_start(bq[:], BQ[m * 128:(m + 1) * 128, :])
                bq_t.append(bq)
            col1 = pool.tile([128, 1], f32, tag="col1")
            nc.vector.memset(col1[:], 1.0)

            # ---- persistent intermediates ----
            qt_sb = [pool.tile([128, T], DT, tag=f"QT{i}", name=f"qt{i}")
                     for i in range(2)]
            kt_sb = [pool.tile([128, T], DT, tag=f"KT{i}", name=f"kt{i}")
                     for i in range(2)]
            v_sb = [pool.tile([128, H_LOC * (D + 1)], DT, tag=f"V{i}",
                              name=f"v{i}") for i in range(NT)]
            ao_sb = [pool.tile([128, T], DT, tag=f"AO{i}", name=f"ao{i}")
                     for i in range(2)]

            # ---- phase 1a: Q^T, K^T (evict on DVE; ACT is Exp-only) ----
            for m in range(4):
                for n in range(T // CQ):
                    ps = psum.tile([128, CQ], f32, tag="prj", bufs=2, name="ps")
                    for kc in range(KC):
                        nc.tensor.matmul(
                            ps[:],
                            w_t[kc][:, m * 128:(m + 1) * 128],
                            a_t[kc][:, n * CQ:(n + 1) * CQ],
                            start=(kc == 0), stop=(kc == KC - 1))
                    if m < 2:
                        nc.vector.tensor_scalar_add(
                            qt_sb[m][:, n * CQ:(n + 1) * CQ], ps[:],
                            bq_t[m][:, 0:1])
                    else:
                        nc.vector.tensor_copy(
                            kt_sb[m - 2][:, n * CQ:(n + 1) * CQ], ps[:])

            # ---- phase 1b: V natural layout ----
            for tt in range(NT):
                ps = psum.tile([128, CW], f32, tag="mm", bufs=2, name="psv")[:, 0:OL]
                for kc in range(KC):
                    nc.tensor.matmul(
                        ps[:],
                        a_t[kc][:, tt * 128:(tt + 1) * 128],
                        w_t[kc][:, 2 * OL:3 * OL],
                        start=(kc == 0), stop=(kc == KC - 1))
                for h in range(H_LOC):
                    nc.vector.tensor_copy(
                        v_sb[tt][:, h * (D + 1):h * (D + 1) + D],
                        ps[:, h * D:(h + 1) * D])
                    nc.vector.tensor_copy(
                        v_sb[tt][:, h * (D + 1) + D:(h + 1) * (D + 1)],
                        col1[:])

            # ---- phase 2+3: causal flash attention, proj interleaved ----
            for qjj in range(NW):              # 1024-wide q window
                for h in range(H_LOC):
                    ht, hp = h // 2, (h % 2) * 64
                    n_kc = (qjj + 1) * (CW // CK)
                    pv0 = psum.tile([D + 1, CQ], f32, tag="pv", bufs=2)
                    pv1 = psum.tile([D + 1, CQ], f32, tag="pv", bufs=2)
                    q0 = qjj * CW              # window start
                    # per-half contraction depths (causal)
                    nk0 = (qjj * 2 + 1) * 4    # half0 needs kc < nk0
                    nk1 = (qjj * 2 + 2) * 4
                    for kc in range(n_kc):
                        # which q-halves need this key chunk
                        use0 = kc < nk0
                        w = CW if use0 else CQ
                        qoff = q0 if use0 else q0 + CQ
                        sp = psum.tile([128, CW], f32, tag="mm", bufs=2)
                        for half in range(2 if use0 else 1):
                            nc.tensor.matmul(
                                sp[:, half * CQ:(half + 1) * CQ],
                                kt_sb[ht][hp:hp + D, kc * CK:(kc + 1) * CK],
                                qt_sb[ht][hp:hp + D,
                                          qoff + half * CQ:qoff + (half + 1) * CQ],
                                start=True, stop=True)
                        pt = wpool.tile([128, CW], DT, tag="pT", bufs=6)
                        nc.scalar.activation(pt[:, 0:w], sp[:, 0:w], Exp,
                                             scale=1.0 / 8.0)
                        # causal mask: at most one diagonal 512-half per kc;
                        # it always sits at pt[:, 0:CQ] (half1-only case is
                        # stored there too). base = qstart - kc*CK.
                        if use0 and kc >= qjj * 8:
                            dbase = q0 - kc * CK          # half0 diagonal
                        elif not use0:
                            dbase = (q0 + CQ) - kc * CK   # half1 diagonal
                        else:
                            dbase = None
                        if dbase is not None:
                            nc.gpsimd.affine_select(
                                out=pt[:, 0:CQ], in_=pt[:, 0:CQ],
                                compare_op=mybir.AluOpType.is_ge,
                                fill=0.0, base=dbase,
                                pattern=[[1, CQ]], channel_multiplier=-1)
                        # PV per 512 half
                        if use0:
                            nc.tensor.matmul(
                                pv0[:],
                                v_sb[kc][:, h * (D + 1):(h + 1) * (D + 1)],
                                pt[:, 0:CQ],
                                start=(kc == 0), stop=(kc == nk0 - 1))
                            nc.tensor.matmul(
                                pv1[:],
                                v_sb[kc][:, h * (D + 1):(h + 1) * (D + 1)],
                                pt[:, CQ:CW],
                                start=(kc == 0), stop=(kc == nk1 - 1))
                        else:
                            nc.tensor.matmul(
                                pv1[:],
                                v_sb[kc][:, h * (D + 1):(h + 1) * (D + 1)],
                                pt[:, 0:CQ],
                                start=(kc == 0), stop=(kc == nk1 - 1))
                    for half, pv in ((0, pv0), (1, pv1)):
                        # approx_fast cannot read PSUM on HW; bounce via SBUF
                        ls = wpool.tile([1, CQ], f32, tag="ls", bufs=2)
                        nc.vector.tensor_copy(ls[:], pv[D:D + 1, :])
                        r = wpool.tile([1, CQ], f32, tag="r", bufs=2)
                        with nc.allow_low_precision(reason="softmax denom"):
                            nc.vector.reciprocal_approx_fast(r[:], ls[:])
                        rbs = wpool.tile([D, CQ], f32, tag="rbs", bufs=2)
                        nc.gpsimd.partition_broadcast(rbs[:], r[:])
                        qs = q0 + half * CQ
                        nc.vector.tensor_mul(
                            ao_sb[ht][hp:hp + D, qs:qs + CQ],
                            pv[0:D, :], rbs[:])

                # proj + store for this window's token tiles (overlaps the
                # next window's attention)
                for tt in range(qjj * (CW // 128), (qjj + 1) * (CW // 128)):
                    for n in range(2):
                        ps = psum.tile([128, CQ], f32, tag="prj", bufs=2,
                                       name="psp")
                        for kc in range(2):
                            nc.tensor.matmul(
                                ps[:],
                                ao_sb[kc][:, tt * 128:(tt + 1) * 128],
                                wp_t[kc][:, n * CQ:(n + 1) * CQ],
                                start=(kc == 0), stop=(kc == 1))
                        yt = wpool.tile([128, CQ], f32, tag="y", bufs=3)
                        nc.vector.tensor_copy(yt[:], ps[:])
                        nc.sync.dma_start(
                            Y[tt * 128:(tt + 1) * 128, n * CQ:(n + 1) * CQ],
                            yt[:])

    nc.compile()
    return nc


def _get_compiled():
    global _COMPILED
    if _COMPILED is None:
        _COMPILED = _build()
    return _COMPILED


def kernel(x, Wq, bq, Wk, bk, Wv, bv, Wp, bp):
    x = np.asarray(x, dtype=np.float32)
    Wq = np.asarray(Wq, dtype=np.float32)
    bq = np.asarray(bq, dtype=np.float32)
    Wk = np.asarray(Wk, dtype=np.float32)
    Wv = np.asarray(Wv, dtype=np.float32)
    Wp = np.asarray(Wp, dtype=np.float32)
    bv = np.asarray(bv, dtype=np.float32)
    bp = np.asarray(bp, dtype=np.float32)

    nc = _get_compiled()

    in_maps = []
    for c in range(N_CORES):
        b, g = divmod(c, 4)
        sl = slice(g * OL, (g + 1) * OL)
        in_maps.append({
            "A": np.ascontiguousarray(x[b].T).astype(NP_DT),
            "Wqkv": np.concatenate([Wq[sl].T, Wk[sl].T, Wv[sl].T], axis=1).astype(NP_DT),
            "WpT": np.ascontiguousarray(Wp[:, sl].T).astype(NP_DT),
            "BQ": bq[sl].reshape(OL, 1),
        })

    res = run_bass_kernel_spmd(nc, in_maps, core_ids=list(range(N_CORES)))

    extra = bv @ Wp.T + bp  # bv/bp fold out of the device kernel
    out = np.empty((B, T, C), dtype=np.float32)
    for b in range(B):
        acc = res.results[4 * b]["Y"].astype(np.float32)
        for g in range(1, 4):
            acc = acc + res.results[4 * b + g]["Y"]
        out[b] = acc + extra
    return out


# revision 18
# speedup vs baseline: 1.1055x; 1.0264x over previous
"""Multi-head causal attention (B=2, T=2048, C=1024, H=16, D=64) on 8 trn2 cores.

Sharding: core c -> batch b = c//4, head group g = c%4 (4 heads each).
Megatron-style: QKV column-parallel, proj row-parallel; partial outputs are
summed on the host (bk is softmax-invariant and dropped; bv/bp fold into a
host-side constant).

Device kernel (per core), matmuls in fp32r:
  A = x[b].T                       [1024, 2048]  (host-transposed)
  Q^T (+bq) / K^T = W.T @ A        [256, 2048]   channels on partitions
  V = A.T @ Wv_loc.T               [2048, 4*(64+1)]  natural layout, a ones
                                   column per head for softmax denominators
  per head h, 1024-wide q-window qjj, 128-key chunk kc:
     S^T[k,q] = K_h^T.T @ Q_h^T    PSUM [128, <=1024]
     P^T = exp(0.125*S^T)          single ACT op per window (ACT runs Exp only)
     diag chunks masked on GpSimd via affine_select
     PV~[65,512] += V~_h[kc].T @ P^T   row 64 accumulates the denominator l
     out^T = PV[0:64] * bcast(1/l)     (approx recip + gpsimd partition_broadcast)
  Y = attn-out^T.T @ Wp_loc.T      [2048, 1024]  partial, summed on host
"""

import sys

sys.path.insert(0, "/opt/trn_rl_repo")

import numpy as np
import ml_dtypes

NP_DT = ml_dtypes.bfloat16

import concourse.bass as bass  # noqa: F401
import concourse.mybir as mybir
import concourse.tile as tile
from concourse import bacc
from concourse.bass_utils import run_bass_kernel_spmd

N_CORES = 8
B, T, C = 2, 2048, 1024
H, D = 16, 64
H_LOC = 4              # heads per core
OL = H_LOC * D         # local channels = 256
CQ = 512               # PSUM-bank q chunk
CW = 1024              # exp window (2 PSUM banks)
CK = 128               # k chunk (partition dim)
NW = T // CW           # 2
NT = T // 128          # 16
KC = C // 128          # 8 contraction chunks for QKV

f32 = mybir.dt.float32
f32r = mybir.dt.float32r
bf16 = mybir.dt.bfloat16
DT = bf16  # matmul operand dtype

_COMPILED = None


def _build():
    nc = bacc.Bacc("TRN2", debug=False, num_devices=N_CORES)

    A = nc.dram_tensor("A", [4 * C, CQ], DT, kind="ExternalInput").ap()
    Wqkv = nc.dram_tensor("Wqkv", [C, 3 * OL], DT, kind="ExternalInput").ap()
    WpT = nc.dram_tensor("WpT", [OL, C], DT, kind="ExternalInput").ap()
    BQ = nc.dram_tensor("BQ", [OL, 1], f32, kind="ExternalInput").ap()
    Y = nc.dram_tensor("Y", [T, C], f32, kind="ExternalOutput").ap()

    Exp = mybir.ActivationFunctionType.Exp

    with tile.TileContext(nc) as tc:
        with tc.tile_pool(name="sbuf", bufs=1) as pool, \
             tc.tile_pool(name="work", bufs=1) as wpool, \
             tc.tile_pool(name="psum", bufs=1, space="PSUM") as psum:

            # ---- resident inputs (piece-contiguous loads, weights first) ----
            a_t, w_t = [], []
            for kc in range(KC):
                at = pool.tile([128, T], DT, tag=f"A{kc}", name=f"a{kc}")
                a_t.append(at)
                wt = pool.tile([128, 3 * OL], DT, tag=f"W{kc}", name=f"w{kc}")
                w_t.append(wt)
                nc.sync.dma_start(wt[:], Wqkv[kc * 128:(kc + 1) * 128, :])
            # A is host-repacked so block (kc, piece) = rows
            # (kc*4+piece)*128..+128 is one contiguous 128KB read; fill
            # token-window 0 first across all kc so QKV starts early
            for piece in range(4):
                for kc in range(KC):
                    blk = (kc * 4 + piece) * 128
                    nc.sync.dma_start(
                        a_t[kc][:, piece * CQ:(piece + 1) * CQ],
                        A[blk:blk + 128, 0:CQ])
            wp_t = []
            for kc in range(2):
                wp = pool.tile([128, C], DT, tag=f"WP{kc}", name=f"wp{kc}")
                nc.sync.dma_start(wp[:], WpT[kc * 128:(kc + 1) * 128, :])
                wp_t.append(wp)
            bq_t = []
            for m in range(2):
                bq = pool.tile([128, 1], f32, tag=f"BQ{m}", name=f"bq{m}")
                nc.sync.dma_start(bq[:], BQ[m * 128:(m + 1) * 128, :])
                bq_t.append(bq)
            col1 = pool.tile([128, 1], f32, tag="col1")
            nc.vector.memset(col1[:], 1.0)

            # ---- persistent intermediates ----
            qt_sb = [pool.tile([128, T], DT, tag=f"QT{i}", name=f"qt{i}")
                     for i in range(2)]
            kt_sb = [pool.tile([128, T], DT, tag=f"KT{i}", name=f"kt{i}")
                     for i in range(2)]
            v_sb = [pool.tile([128, H_LOC * (D + 1)], DT, tag=f"V{i}",
                              name=f"v{i}") for i in range(NT)]
            ao_sb = [pool.tile([128, T], DT, tag=f"AO{i}", name=f"ao{i}")
                     for i in range(2)]

            # ---- phase 1a: Q^T, K^T (evict on DVE; ACT is Exp-only) ----
            for m in range(4):
                for n in range(T // CQ):
                    ps = psum.tile([128, CQ], f32, tag="prj", bufs=2, name="ps")
                    for kc in range(KC):
                        nc.tensor.matmul(
                            ps[:],
                            w_t[kc][:, m * 128:(m + 1) * 128],
                            a_t[kc][:, n * CQ:(n + 1) * CQ],
                            start=(kc == 0), stop=(kc == KC - 1))
                    if m < 2:
                        nc.vector.tensor_scalar_add(
                            qt_sb[m][:, n * CQ:(n + 1) * CQ], ps[:],
                            bq_t[m][:, 0:1])
                    else:
                        nc.vector.tensor_copy(
                            kt_sb[m - 2][:, n * CQ:(n + 1) * CQ], ps[:])

            # ---- phase 1b: V natural layout ----
            for tt in range(NT):
                ps = psum.tile([128, CW], f32, tag="mm", bufs=2, name="psv")[:, 0:OL]
                for kc in range(KC):
                    nc.tensor.matmul(
                        ps[:],
                        a_t[kc][:, tt * 128:(tt + 1) * 128],
                        w_t[kc][:, 2 * OL:3 * OL],
                        start=(kc == 0), stop=(kc == KC - 1))
                for h in range(H_LOC):
                    nc.vector.tensor_copy(
                        v_sb[tt][:, h * (D + 1):h * (D + 1) + D],
                        ps[:, h * D:(h + 1) * D])
                    nc.vector.tensor_copy(
                        v_sb[tt][:, h * (D + 1) + D:(h + 1) * (D + 1)],
                        col1[:])

            # ---- phase 2+3: causal flash attention, proj interleaved ----
            for qjj in range(NW):              # 1024-wide q window
                for h in range(H_LOC):
                    ht, hp = h // 2, (h % 2) * 64
                    n_kc = (qjj + 1) * (CW // CK)
                    pv0 = psum.tile([D + 1, CQ], f32, tag="pv", bufs=2)
                    pv1 = psum.tile([D + 1, CQ], f32, tag="pv", bufs=2)
                    q0 = qjj * CW              # window start
                    # per-half contraction depths (causal)
                    nk0 = (qjj * 2 + 1) * 4    # half0 needs kc < nk0
                    nk1 = (qjj * 2 + 2) * 4
                    for kc in range(n_kc):
                        # which q-halves need this key chunk
                        use0 = kc < nk0
                        w = CW if use0 else CQ
                        qoff = q0 if use0 else q0 + CQ
                        sp = psum.tile([128, CW], f32, tag="mm", bufs=2)
                        for half in range(2 if use0 else 1):
                            nc.tensor.matmul(
                                sp[:, half * CQ:(half + 1) * CQ],
                                kt_sb[ht][hp:hp + D, kc * CK:(kc + 1) * CK],
                                qt_sb[ht][hp:hp + D,
                                          qoff + half * CQ:qoff + (half + 1) * CQ],
                                start=True, stop=True)
                        pt = wpool.tile([128, CW], DT, tag="pT", bufs=6)
                        nc.scalar.activation(pt[:, 0:w], sp[:, 0:w], Exp,
                                             scale=1.0 / 8.0)
                        # causal mask: at most one diagonal 512-half per kc;
                        # it always sits at pt[:, 0:CQ] (half1-only case is
                        # stored there too). base = qstart - kc*CK.
                        if use0 and kc >= qjj * 8:
                            dbase = q0 - kc * CK          # half0 diagonal
                        elif not use0:
                            dbase = (q0 + CQ) - kc * CK   # half1 diagonal
                        else:
                            dbase = None
                        if dbase is not None:
                            nc.gpsimd.affine_select(
                                out=pt[:, 0:CQ], in_=pt[:, 0:CQ],
                                compare_op=mybir.AluOpType.is_ge,
                                fill=0.0, base=dbase,
                                pattern=[[1, CQ]], channel_multiplier=-1)
                        # PV per 512 half
                        if use0:
                            nc.tensor.matmul(
                                pv0[:],
                                v_sb[kc][:, h * (D + 1):(h + 1) * (D + 1)],
                                pt[:, 0:CQ],
                                start=(kc == 0), stop=(kc == nk0 - 1))
                            nc.tensor.matmul(
                                pv1[:],
                                v_sb[kc][:, h * (D + 1):(h + 1) * (D + 1)],
                                pt[:, CQ:CW],
                                start=(kc == 0), stop=(kc == nk1 - 1))
                        else:
                            nc.tensor.matmul(
                                pv1[:],
                                v_sb[kc][:, h * (D + 1):(h + 1) * (D + 1)],
                                pt[:, 0:CQ],
                                start=(kc == 0), stop=(kc == nk1 - 1))
                    for half, pv in ((0, pv0), (1, pv1)):
                        # approx_fast cannot read PSUM on HW; bounce via SBUF
                        ls = wpool.tile([1, CQ], f32, tag="ls", bufs=2)
                        nc.vector.tensor_copy(ls[:], pv[D:D + 1, :])
                        r = wpool.tile([1, CQ], f32, tag="r", bufs=2)
                        with nc.allow_low_precision(reason="softmax denom"):
                            nc.vector.reciprocal_approx_fast(r[:], ls[:])
                        rbs = wpool.tile([D, CQ], f32, tag="rbs", bufs=2)
                        nc.gpsimd.partition_broadcast(rbs[:], r[:])
                        qs = q0 + half * CQ
                        nc.vector.tensor_mul(
                            ao_sb[ht][hp:hp + D, qs:qs + CQ],
                            pv[0:D, :], rbs[:])

                # proj + store for this window's token tiles (overlaps the
                # next window's attention)
                for tt in range(qjj * (CW // 128), (qjj + 1) * (CW // 128)):
                    for n in range(2):
                        ps = psum.tile([128, CQ], f32, tag="prj", bufs=2,
                                       name="psp")
                        for kc in range(2):
                            nc.tensor.matmul(
                                ps[:],
                                ao_sb[kc][:, tt * 128:(tt + 1) * 128],
                                wp_t[kc][:, n * CQ:(n + 1) * CQ],
                                start=(kc == 0), stop=(kc == 1))
                        yt = wpool.tile([128, CQ], f32, tag="y", bufs=3)
                        nc.vector.tensor_copy(yt[:], ps[:])
                        nc.sync.dma_start(
                            Y[tt * 128:(tt + 1) * 128, n * CQ:(n + 1) * CQ],
                            yt[:])

    nc.compile()
    return nc


def _get_compiled():
    global _COMPILED
    if _COMPILED is None:
        _COMPILED = _build()
    return _COMPILED


def make_in_maps(x, Wq, bq, Wk, Wv, Wp):
    in_maps = []
    for c in range(N_CORES):
        b, g = divmod(c, 4)
        sl = slice(g * OL, (g + 1) * OL)
        in_maps.append({
            "A": np.ascontiguousarray(
                x[b].T.reshape(KC, 128, 4, CQ).transpose(0, 2, 1, 3)
                .reshape(4 * C, CQ)).astype(NP_DT),
            "Wqkv": np.concatenate(
                [Wq[sl].T, Wk[sl].T, Wv[sl].T], axis=1).astype(NP_DT),
            "WpT": np.ascontiguousarray(Wp[:, sl].T).astype(NP_DT),
            "BQ": bq[sl].reshape(OL, 1).astype(np.float32),
        })
    return in_maps


def kernel(x, Wq, bq, Wk, bk, Wv, bv, Wp, bp):
    x = np.asarray(x, dtype=np.float32)
    Wq = np.asarray(Wq, dtype=np.float32)
    bq = np.asarray(bq, dtype=np.float32)
    Wk = np.asarray(Wk, dtype=np.float32)
    Wv = np.asarray(Wv, dtype=np.float32)
    Wp = np.asarray(Wp, dtype=np.float32)
    bv = np.asarray(bv, dtype=np.float32)
    bp = np.asarray(bp, dtype=np.float32)

    nc = _get_compiled()

    in_maps = make_in_maps(x, Wq, bq, Wk, Wv, Wp)

    res = run_bass_kernel_spmd(nc, in_maps, core_ids=list(range(N_CORES)))

    extra = bv @ Wp.T + bp  # bv/bp fold out of the device kernel
    out = np.empty((B, T, C), dtype=np.float32)
    for b in range(B):
        acc = res.results[4 * b]["Y"].astype(np.float32)
        for g in range(1, 4):
            acc = acc + res.results[4 * b + g]["Y"]
        out[b] = acc + extra
    return out


# revision 20
# speedup vs baseline: 1.1199x; 1.0130x over previous
"""Multi-head causal attention (B=2, T=2048, C=1024, H=16, D=64) on 8 trn2 cores.

Sharding: core c -> batch b = c//4, head group g = c%4 (4 heads each).
Megatron-style: QKV column-parallel, proj row-parallel; partial outputs are
summed on the host (bk is softmax-invariant and dropped; bv/bp fold into a
host-side constant).

Device kernel (per core), matmuls in fp32r:
  A = x[b].T                       [1024, 2048]  (host-transposed)
  Q^T (+bq) / K^T = W.T @ A        [256, 2048]   channels on partitions
  V = A.T @ Wv_loc.T               [2048, 4*(64+1)]  natural layout, a ones
                                   column per head for softmax denominators
  per head h, 1024-wide q-window qjj, 128-key chunk kc:
     S^T[k,q] = K_h^T.T @ Q_h^T    PSUM [128, <=1024]
     P^T = exp(0.125*S^T)          single ACT op per window (ACT runs Exp only)
     diag chunks masked on GpSimd via affine_select
     PV~[65,512] += V~_h[kc].T @ P^T   row 64 accumulates the denominator l
     out^T = PV[0:64] * bcast(1/l)     (approx recip + gpsimd partition_broadcast)
  Y = attn-out^T.T @ Wp_loc.T      [2048, 1024]  partial, summed on host
"""

import sys

sys.path.insert(0, "/opt/trn_rl_repo")

import numpy as np
import ml_dtypes

NP_DT = ml_dtypes.bfloat16

import concourse.bass as bass  # noqa: F401
import concourse.mybir as mybir
import concourse.tile as tile
from concourse import bacc
from concourse.bass_utils import run_bass_kernel_spmd

N_CORES = 8
B, T, C = 2, 2048, 1024
H, D = 16, 64
H_LOC = 4              # heads per core
OL = H_LOC * D         # local channels = 256
CQ = 512               # PSUM-bank q chunk
CW = 1024              # exp window (2 PSUM banks)
CK = 128               # k chunk (partition dim)
NW = T // CW           # 2
NT = T // 128          # 16
KC = C // 128          # 8 contraction chunks for QKV

f32 = mybir.dt.float32
f32r = mybir.dt.float32r
bf16 = mybir.dt.bfloat16
DT = bf16  # matmul operand dtype

_COMPILED = None


def _build():
    nc = bacc.Bacc("TRN2", debug=False, num_devices=N_CORES)

    A = nc.dram_tensor("A", [4 * C, CQ], DT, kind="ExternalInput").ap()
    Wqkv = nc.dram_tensor("Wqkv", [C, 3 * OL], DT, kind="ExternalInput").ap()
    WpT = nc.dram_tensor("WpT", [OL, C], DT, kind="ExternalInput").ap()
    BQ = nc.dram_tensor("BQ", [OL, 1], f32, kind="ExternalInput").ap()
    Y = nc.dram_tensor("Y", [T, C], f32, kind="ExternalOutput").ap()

    Exp = mybir.ActivationFunctionType.Exp

    with tile.TileContext(nc) as tc:
        with tc.tile_pool(name="sbuf", bufs=1) as pool, \
             tc.tile_pool(name="work", bufs=1) as wpool, \
             tc.tile_pool(name="psum", bufs=1, space="PSUM") as psum:

            # ---- resident inputs (piece-contiguous loads, weights first) ----
            a_t, w_t = [], []
            for kc in range(KC):
                at = pool.tile([128, T], DT, tag=f"A{kc}", name=f"a{kc}")
                a_t.append(at)
                wt = pool.tile([128, 3 * OL], DT, tag=f"W{kc}", name=f"w{kc}")
                w_t.append(wt)
                nc.sync.dma_start(wt[:], Wqkv[kc * 128:(kc + 1) * 128, :])
            # A is host-repacked so block (kc, piece) = rows
            # (kc*4+piece)*128..+128 is one contiguous 128KB read; fill
            # token-window 0 first across all kc so QKV starts early
            for piece in range(4):
                for kc in range(KC):
                    blk = (kc * 4 + piece) * 128
                    nc.sync.dma_start(
                        a_t[kc][:, piece * CQ:(piece + 1) * CQ],
                        A[blk:blk + 128, 0:CQ])
            wp_t = []
            for kc in range(2):
                wp = pool.tile([128, C], DT, tag=f"WP{kc}", name=f"wp{kc}")
                nc.sync.dma_start(wp[:], WpT[kc * 128:(kc + 1) * 128, :])
                wp_t.append(wp)
            bq_t = []
            for m in range(2):
                bq = pool.tile([128, 1], f32, tag=f"BQ{m}", name=f"bq{m}")
                nc.sync.dma_start(bq[:], BQ[m * 128:(m + 1) * 128, :])
                bq_t.append(bq)
            col1 = pool.tile([128, 1], f32, tag="col1")
            nc.vector.memset(col1[:], 1.0)

            # ---- persistent intermediates ----
            qt_sb = [pool.tile([128, T], DT, tag=f"QT{i}", name=f"qt{i}")
                     for i in range(2)]
            kt_sb = [pool.tile([128, T], DT, tag=f"KT{i}", name=f"kt{i}")
                     for i in range(2)]
            v_sb = [pool.tile([128, H_LOC * (D + 1)], DT, tag=f"V{i}",
                              name=f"v{i}") for i in range(NT)]
            ao_sb = [pool.tile([128, T], DT, tag=f"AO{i}", name=f"ao{i}")
                     for i in range(2)]

            # ---- phase 1a: Q^T, K^T (evict on DVE; ACT is Exp-only) ----
            for m in range(4):
                for n in range(T // CQ):
                    ps = psum.tile([128, CQ], f32, tag="prj", bufs=2, name="ps")
                    for kc in range(KC):
                        nc.tensor.matmul(
                            ps[:],
                            w_t[kc][:, m * 128:(m + 1) * 128],
                            a_t[kc][:, n * CQ:(n + 1) * CQ],
                            start=(kc == 0), stop=(kc == KC - 1))
                    if m < 2:
                        nc.vector.tensor_scalar_add(
                            qt_sb[m][:, n * CQ:(n + 1) * CQ], ps[:],
                            bq_t[m][:, 0:1])
                    else:
                        nc.vector.tensor_copy(
                            kt_sb[m - 2][:, n * CQ:(n + 1) * CQ], ps[:])

            # ---- phase 1b: V natural layout ----
            for tt in range(NT):
                ps = psum.tile([128, CQ], f32, tag="mm", bufs=4, name="psv")[:, 0:OL]
                for kc in range(KC):
                    nc.tensor.matmul(
                        ps[:],
                        a_t[kc][:, tt * 128:(tt + 1) * 128],
                        w_t[kc][:, 2 * OL:3 * OL],
                        start=(kc == 0), stop=(kc == KC - 1))
                for h in range(H_LOC):
                    nc.vector.tensor_copy(
                        v_sb[tt][:, h * (D + 1):h * (D + 1) + D],
                        ps[:, h * D:(h + 1) * D])
                    nc.vector.tensor_copy(
                        v_sb[tt][:, h * (D + 1) + D:(h + 1) * (D + 1)],
                        col1[:])

            # ---- phase 2+3: causal flash attention, proj interleaved ----
            # Per (h, qj): S chunks software-pipelined 2 ahead of PV so the
            # PE (in-order queue) never waits on the ACT exp; diagonal
            # chunks first so the GpSimd mask latency hides behind the
            # remaining S matmuls.
            for qj in range(T // CQ):          # 512-wide q chunk
                q0 = qj * CQ
                for h in range(H_LOC):
                    ht, hp = h // 2, (h % 2) * 64
                    n_kc = (qj + 1) * (CQ // CK)
                    order = list(range(qj * 4, n_kc)) + list(range(0, qj * 4))
                    pv = psum.tile([D + 1, CQ], f32, tag="pv", bufs=2)
                    pts = {}

                    def emit_s(kc, ht=ht, hp=hp, q0=q0, qj=qj, pts=pts):
                        sp = psum.tile([128, CQ], f32, tag="mm", bufs=4)
                        nc.tensor.matmul(
                            sp[:],
                            kt_sb[ht][hp:hp + D, kc * CK:(kc + 1) * CK],
                            qt_sb[ht][hp:hp + D, q0:q0 + CQ],
                            start=True, stop=True)
                        pt = wpool.tile([128, CQ], DT, tag="pT", bufs=8)
                        nc.scalar.activation(pt[:], sp[:], Exp, scale=1.0 / 8.0)
                        if kc >= qj * 4:   # diagonal chunk: mask q < k
                            nc.gpsimd.affine_select(
                                out=pt[:], in_=pt[:],
                                compare_op=mybir.AluOpType.is_ge,
                                fill=0.0, base=q0 - kc * CK,
                                pattern=[[1, CQ]], channel_multiplier=-1)
                        pts[kc] = pt

                    for j in range(min(2, n_kc)):
                        emit_s(order[j])
                    for i, kc in enumerate(order):
                        if i + 2 < n_kc:
                            emit_s(order[i + 2])
                        nc.tensor.matmul(
                            pv[:],
                            v_sb[kc][:, h * (D + 1):(h + 1) * (D + 1)],
                            pts.pop(kc),
                            start=(i == 0), stop=(i == n_kc - 1))
                    # normalize: approx recip of the ones-row, broadcast, mul
                    ls = wpool.tile([1, CQ], f32, tag="ls", bufs=2)
                    nc.vector.tensor_copy(ls[:], pv[D:D + 1, :])
                    r = wpool.tile([1, CQ], f32, tag="r", bufs=2)
                    with nc.allow_low_precision(reason="softmax denom"):
                        nc.vector.reciprocal_approx_fast(r[:], ls[:])
                    rbs = wpool.tile([D, CQ], f32, tag="rbs", bufs=2)
                    nc.gpsimd.partition_broadcast(rbs[:], r[:])
                    nc.vector.tensor_mul(
                        ao_sb[ht][hp:hp + D, q0:q0 + CQ],
                        pv[0:D, :], rbs[:])

                # proj + store for this chunk's token tiles (overlaps the
                # next chunk's attention)
                for tt in range(qj * (CQ // 128), (qj + 1) * (CQ // 128)):
                    for n in range(2):
                        ps = psum.tile([128, CQ], f32, tag="prj", bufs=2,
                                       name="psp")
                        for kc in range(2):
                            nc.tensor.matmul(
                                ps[:],
                                ao_sb[kc][:, tt * 128:(tt + 1) * 128],
                                wp_t[kc][:, n * CQ:(n + 1) * CQ],
                                start=(kc == 0), stop=(kc == 1))
                        yt = wpool.tile([128, CQ], f32, tag="y", bufs=3)
                        nc.vector.tensor_copy(yt[:], ps[:])
                        nc.sync.dma_start(
                            Y[tt * 128:(tt + 1) * 128, n * CQ:(n + 1) * CQ],
                            yt[:])

    nc.compile()
    return nc


def _get_compiled():
    global _COMPILED
    if _COMPILED is None:
        _COMPILED = _build()
    return _COMPILED


def make_in_maps(x, Wq, bq, Wk, Wv, Wp):
    in_maps = []
    for c in range(N_CORES):
        b, g = divmod(c, 4)
        sl = slice(g * OL, (g + 1) * OL)
        in_maps.append({
            "A": np.ascontiguousarray(
                x[b].T.reshape(KC, 128, 4, CQ).transpose(0, 2, 1, 3)
                .reshape(4 * C, CQ)).astype(NP_DT),
            "Wqkv": np.concatenate(
                [Wq[sl].T, Wk[sl].T, Wv[sl].T], axis=1).astype(NP_DT),
            "WpT": np.ascontiguousarray(Wp[:, sl].T).astype(NP_DT),
            "BQ": bq[sl].reshape(OL, 1).astype(np.float32),
        })
    return in_maps


def kernel(x, Wq, bq, Wk, bk, Wv, bv, Wp, bp):
    x = np.asarray(x, dtype=np.float32)
    Wq = np.asarray(Wq, dtype=np.float32)
    bq = np.asarray(bq, dtype=np.float32)
    Wk = np.asarray(Wk, dtype=np.float32)
    Wv = np.asarray(Wv, dtype=np.float32)
    Wp = np.asarray(Wp, dtype=np.float32)
    bv = np.asarray(bv, dtype=np.float32)
    bp = np.asarray(bp, dtype=np.float32)

    nc = _get_compiled()

    in_maps = make_in_maps(x, Wq, bq, Wk, Wv, Wp)

    res = run_bass_kernel_spmd(nc, in_maps, core_ids=list(range(N_CORES)))

    extra = bv @ Wp.T + bp  # bv/bp fold out of the device kernel
    out = np.empty((B, T, C), dtype=np.float32)
    for b in range(B):
        acc = res.results[4 * b]["Y"].astype(np.float32)
        for g in range(1, 4):
            acc = acc + res.results[4 * b + g]["Y"]
        out[b] = acc + extra
    return out


# revision 22
# speedup vs baseline: 1.1381x; 1.0162x over previous
"""Multi-head causal attention (B=2, T=2048, C=1024, H=16, D=64) on 8 trn2 cores.

Sharding: core c -> batch b = c//4, head group g = c%4 (4 heads each).
Megatron-style: QKV column-parallel, proj row-parallel; partial outputs are
summed on the host (bk is softmax-invariant and dropped; bv/bp fold into a
host-side constant).

Device kernel (per core), matmuls in fp32r:
  A = x[b].T                       [1024, 2048]  (host-transposed)
  Q^T (+bq) / K^T = W.T @ A        [256, 2048]   channels on partitions
  V = A.T @ Wv_loc.T               [2048, 4*(64+1)]  natural layout, a ones
                                   column per head for softmax denominators
  per head h, 1024-wide q-window qjj, 128-key chunk kc:
     S^T[k,q] = K_h^T.T @ Q_h^T    PSUM [128, <=1024]
     P^T = exp(0.125*S^T)          single ACT op per window (ACT runs Exp only)
     diag chunks masked on GpSimd via affine_select
     PV~[65,512] += V~_h[kc].T @ P^T   row 64 accumulates the denominator l
     out^T = PV[0:64] * bcast(1/l)     (approx recip + gpsimd partition_broadcast)
  Y = attn-out^T.T @ Wp_loc.T      [2048, 1024]  partial, summed on host
"""

import sys

sys.path.insert(0, "/opt/trn_rl_repo")

import numpy as np
import ml_dtypes

NP_DT = ml_dtypes.bfloat16

import concourse.bass as bass  # noqa: F401
import concourse.mybir as mybir
import concourse.tile as tile
from concourse import bacc
from concourse.bass_utils import run_bass_kernel_spmd

N_CORES = 8
B, T, C = 2, 2048, 1024
H, D = 16, 64
H_LOC = 4              # heads per core
OL = H_LOC * D         # local channels = 256
CQ = 512               # PSUM-bank q chunk
CW = 1024              # exp window (2 PSUM banks)
CK = 128               # k chunk (partition dim)
NW = T // CW           # 2
NT = T // 128          # 16
KC = C // 128          # 8 contraction chunks for QKV

f32 = mybir.dt.float32
f32r = mybir.dt.float32r
bf16 = mybir.dt.bfloat16
DT = bf16  # matmul operand dtype

_COMPILED = None


def _build():
    nc = bacc.Bacc("TRN2", debug=False, num_devices=N_CORES)

    A = nc.dram_tensor("A", [4 * C, CQ], DT, kind="ExternalInput").ap()
    Wqkv = nc.dram_tensor("Wqkv", [C, 3 * OL], DT, kind="ExternalInput").ap()
    WpT = nc.dram_tensor("WpT", [OL, C], DT, kind="ExternalInput").ap()
    BQ = nc.dram_tensor("BQ", [OL, 1], f32, kind="ExternalInput").ap()
    Y = nc.dram_tensor("Y", [T, C], f32, kind="ExternalOutput").ap()

    Exp = mybir.ActivationFunctionType.Exp

    with tile.TileContext(nc) as tc:
        with tc.tile_pool(name="sbuf", bufs=1) as pool, \
             tc.tile_pool(name="work", bufs=1) as wpool, \
             tc.tile_pool(name="psum", bufs=1, space="PSUM") as psum:

            # ---- resident inputs (piece-contiguous loads, weights first) ----
            a_t, w_t = [], []
            for kc in range(KC):
                at = pool.tile([128, T], DT, tag=f"A{kc}", name=f"a{kc}")
                a_t.append(at)
                wt = pool.tile([128, 3 * OL], DT, tag=f"W{kc}", name=f"w{kc}")
                w_t.append(wt)
            # A is host-repacked so block (kc, piece) = rows
            # (kc*4+piece)*128..+128 is one contiguous 128KB read. Load
            # w[kc] + piece-0 of a[kc] interleaved so the first QKV
            # accumulation chain (needs all kc) completes earliest.
            for kc in range(KC):
                nc.sync.dma_start(w_t[kc][:], Wqkv[kc * 128:(kc + 1) * 128, :])
                blk = kc * 4 * 128
                nc.sync.dma_start(a_t[kc][:, 0:CQ], A[blk:blk + 128, 0:CQ])
            for piece in range(1, 4):
                for kc in range(KC):
                    blk = (kc * 4 + piece) * 128
                    nc.sync.dma_start(
                        a_t[kc][:, piece * CQ:(piece + 1) * CQ],
                        A[blk:blk + 128, 0:CQ])
            wp_t = []
            for kc in range(2):
                wp = pool.tile([128, C], DT, tag=f"WP{kc}", name=f"wp{kc}")
                nc.sync.dma_start(wp[:], WpT[kc * 128:(kc + 1) * 128, :])
                wp_t.append(wp)
            bq_t = []
            for m in range(2):
                bq = pool.tile([128, 1], f32, tag=f"BQ{m}", name=f"bq{m}")
                nc.sync.dma_start(bq[:], BQ[m * 128:(m + 1) * 128, :])
                bq_t.append(bq)
            col1 = pool.tile([128, 1], f32, tag="col1")
            nc.vector.memset(col1[:], 1.0)
            # warm the GpSimd ucode paths so the first real mask/broadcast
            # doesn't eat the cold-start cost mid-attention
            warm = wpool.tile([128, 8], f32, tag="warm")
            nc.vector.memset(warm[:], 1.0)
            nc.gpsimd.affine_select(
                out=warm[:], in_=warm[:],
                compare_op=mybir.AluOpType.is_ge, fill=0.0, base=0,
                pattern=[[1, 8]], channel_multiplier=-1)
            warm2 = wpool.tile([128, 8], f32, tag="warm2")
            nc.gpsimd.partition_broadcast(warm2[:], warm[0:1, :])

            # ---- persistent intermediates ----
            qt_sb = [pool.tile([128, T], DT, tag=f"QT{i}", name=f"qt{i}")
                     for i in range(2)]
            kt_sb = [pool.tile([128, T], DT, tag=f"KT{i}", name=f"kt{i}")
                     for i in range(2)]
            v_sb = [pool.tile([128, H_LOC * (D + 1)], DT, tag=f"V{i}",
                              name=f"v{i}") for i in range(NT)]
            ao_sb = [pool.tile([128, T], DT, tag=f"AO{i}", name=f"ao{i}")
                     for i in range(2)]

            # ---- phase 1a: Q^T, K^T (evict on DVE; ACT is Exp-only) ----
            for m in range(4):
                for n in range(T // CQ):
                    ps = psum.tile([128, CQ], f32, tag="prj", bufs=2, name="ps")
                    for kc in range(KC):
                        nc.tensor.matmul(
                            ps[:],
                            w_t[kc][:, m * 128:(m + 1) * 128],
                            a_t[kc][:, n * CQ:(n + 1) * CQ],
                            start=(kc == 0), stop=(kc == KC - 1))
                    if m < 2:
                        nc.vector.tensor_scalar_add(
                            qt_sb[m][:, n * CQ:(n + 1) * CQ], ps[:],
                            bq_t[m][:, 0:1])
                    else:
                        nc.vector.tensor_copy(
                            kt_sb[m - 2][:, n * CQ:(n + 1) * CQ], ps[:])

            # ---- phase 1b: V natural layout ----
            for tt in range(NT):
                ps = psum.tile([128, CQ], f32, tag="mm", bufs=4, name="psv")[:, 0:OL]
                for kc in range(KC):
                    nc.tensor.matmul(
                        ps[:],
                        a_t[kc][:, tt * 128:(tt + 1) * 128],
                        w_t[kc][:, 2 * OL:3 * OL],
                        start=(kc == 0), stop=(kc == KC - 1))
                for h in range(H_LOC):
                    nc.vector.tensor_copy(
                        v_sb[tt][:, h * (D + 1):h * (D + 1) + D],
                        ps[:, h * D:(h + 1) * D])
                    nc.vector.tensor_copy(
                        v_sb[tt][:, h * (D + 1) + D:(h + 1) * (D + 1)],
                        col1[:])

            # ---- phase 2+3: causal flash attention, proj interleaved ----
            # Per (h, qj): S chunks software-pipelined 2 ahead of PV so the
            # PE (in-order queue) never waits on the ACT exp; diagonal
            # chunks first so the GpSimd mask latency hides behind the
            # remaining S matmuls.
            for qj in range(T // CQ):          # 512-wide q chunk
                q0 = qj * CQ
                for h in range(H_LOC):
                    ht, hp = h // 2, (h % 2) * 64
                    n_kc = (qj + 1) * (CQ // CK)
                    order = list(range(qj * 4, n_kc)) + list(range(0, qj * 4))
                    pv = psum.tile([D + 1, CQ], f32, tag="pv", bufs=2)
                    pts = {}

                    def emit_s(kc, ht=ht, hp=hp, q0=q0, qj=qj, pts=pts):
                        sp = psum.tile([128, CQ], f32, tag="mm", bufs=4)
                        nc.tensor.matmul(
                            sp[:],
                            kt_sb[ht][hp:hp + D, kc * CK:(kc + 1) * CK],
                            qt_sb[ht][hp:hp + D, q0:q0 + CQ],
                            start=True, stop=True)
                        pt = wpool.tile([128, CQ], DT, tag="pT", bufs=8)
                        nc.scalar.activation(pt[:], sp[:], Exp, scale=1.0 / 8.0)
                        if kc >= qj * 4:   # diagonal chunk: mask q < k
                            nc.gpsimd.affine_select(
                                out=pt[:], in_=pt[:],
                                compare_op=mybir.AluOpType.is_ge,
                                fill=0.0, base=q0 - kc * CK,
                                pattern=[[1, CQ]], channel_multiplier=-1)
                        pts[kc] = pt

                    for j in range(min(3, n_kc)):
                        emit_s(order[j])
                    for i, kc in enumerate(order):
                        if i + 3 < n_kc:
                            emit_s(order[i + 3])
                        nc.tensor.matmul(
                            pv[:],
                            v_sb[kc][:, h * (D + 1):(h + 1) * (D + 1)],
                            pts.pop(kc),
                            start=(i == 0), stop=(i == n_kc - 1))
                    # normalize: approx recip of the ones-row, broadcast, mul
                    ls = wpool.tile([1, CQ], f32, tag="ls", bufs=2)
                    nc.vector.tensor_copy(ls[:], pv[D:D + 1, :])
                    r = wpool.tile([1, CQ], f32, tag="r", bufs=2)
                    with nc.allow_low_precision(reason="softmax denom"):
                        nc.vector.reciprocal_approx_fast(r[:], ls[:])
                    rbs = wpool.tile([D, CQ], f32, tag="rbs", bufs=2)
                    nc.gpsimd.partition_broadcast(rbs[:], r[:])
                    nc.vector.tensor_mul(
                        ao_sb[ht][hp:hp + D, q0:q0 + CQ],
                        pv[0:D, :], rbs[:])

                # proj + store for this chunk's token tiles (overlaps the
                # next chunk's attention)
                for tt in range(qj * (CQ // 128), (qj + 1) * (CQ // 128)):
                    for n in range(2):
                        ps = psum.tile([128, CQ], f32, tag="prj", bufs=2,
                                       name="psp")
                        for kc in range(2):
                            nc.tensor.matmul(
                                ps[:],
                                ao_sb[kc][:, tt * 128:(tt + 1) * 128],
                                wp_t[kc][:, n * CQ:(n + 1) * CQ],
                                start=(kc == 0), stop=(kc == 1))
                        yt = wpool.tile([128, CQ], f32, tag="y", bufs=3)
                        nc.vector.tensor_copy(yt[:], ps[:])
                        nc.sync.dma_start(
                            Y[tt * 128:(tt + 1) * 128, n * CQ:(n + 1) * CQ],
                            yt[:])

    nc.compile()
    return nc


def _get_compiled():
    global _COMPILED
    if _COMPILED is None:
        _COMPILED = _build()
    return _COMPILED


def make_in_maps(x, Wq, bq, Wk, Wv, Wp):
    in_maps = []
    for c in range(N_CORES):
        b, g = divmod(c, 4)
        sl = slice(g * OL, (g + 1) * OL)
        in_maps.append({
            "A": np.ascontiguousarray(
                x[b].T.reshape(KC, 128, 4, CQ).transpose(0, 2, 1, 3)
                .reshape(4 * C, CQ)).astype(NP_DT),
            "Wqkv": np.concatenate(
                [Wq[sl].T, Wk[sl].T, Wv[sl].T], axis=1).astype(NP_DT),
            "WpT": np.ascontiguousarray(Wp[:, sl].T).astype(NP_DT),
            "BQ": bq[sl].reshape(OL, 1).astype(np.float32),
        })
    return in_maps


def kernel(x, Wq, bq, Wk, bk, Wv, bv, Wp, bp):
    x = np.asarray(x, dtype=np.float32)
    Wq = np.asarray(Wq, dtype=np.float32)
    bq = np.asarray(bq, dtype=np.float32)
    Wk = np.asarray(Wk, dtype=np.float32)
    Wv = np.asarray(Wv, dtype=np.float32)
    Wp = np.asarray(Wp, dtype=np.float32)
    bv = np.asarray(bv, dtype=np.float32)
    bp = np.asarray(bp, dtype=np.float32)

    nc = _get_compiled()

    in_maps = make_in_maps(x, Wq, bq, Wk, Wv, Wp)

    res = run_bass_kernel_spmd(nc, in_maps, core_ids=list(range(N_CORES)))

    extra = bv @ Wp.T + bp  # bv/bp fold out of the device kernel
    out = np.empty((B, T, C), dtype=np.float32)
    for b in range(B):
        acc = res.results[4 * b]["Y"].astype(np.float32)
        for g in range(1, 4):
            acc = acc + res.results[4 * b + g]["Y"]
        out[b] = acc + extra
    return out


# revision 24
# speedup vs baseline: 1.2525x; 1.1006x over previous
"""Multi-head causal attention (B=2, T=2048, C=1024, H=16, D=64) on 8 trn2 cores.

Sharding: core c -> batch b = c//4, head group g = c%4 (4 heads each).
Megatron-style: QKV column-parallel, proj row-parallel; partial outputs are
summed on the host (bk is softmax-invariant and dropped; bv/bp fold into a
host-side constant).

Device kernel (per core), matmuls in fp32r:
  A = x[b].T                       [1024, 2048]  (host-transposed)
  Q^T (+bq) / K^T = W.T @ A        [256, 2048]   channels on partitions
  V = A.T @ Wv_loc.T               [2048, 4*(64+1)]  natural layout, a ones
                                   column per head for softmax denominators
  per head h, 1024-wide q-window qjj, 128-key chunk kc:
     S^T[k,q] = K_h^T.T @ Q_h^T    PSUM [128, <=1024]
     P^T = exp(0.125*S^T)          single ACT op per window (ACT runs Exp only)
     diag chunks masked on GpSimd via affine_select
     PV~[65,512] += V~_h[kc].T @ P^T   row 64 accumulates the denominator l
     out^T = PV[0:64] * bcast(1/l)     (approx recip + gpsimd partition_broadcast)
  Y = attn-out^T.T @ Wp_loc.T      [2048, 1024]  partial, summed on host
"""

import sys

sys.path.insert(0, "/opt/trn_rl_repo")

import numpy as np
import ml_dtypes

NP_DT = ml_dtypes.bfloat16

import concourse.bass as bass  # noqa: F401
import concourse.mybir as mybir
import concourse.tile as tile
from concourse import bacc
from concourse.bass_utils import run_bass_kernel_spmd

N_CORES = 8
B, T, C = 2, 2048, 1024
H, D = 16, 64
H_LOC = 4              # heads per core
OL = H_LOC * D         # local channels = 256
CQ = 512               # PSUM-bank q chunk
CW = 1024              # exp window (2 PSUM banks)
CK = 128               # k chunk (partition dim)
NW = T // CW           # 2
NT = T // 128          # 16
KC = C // 128          # 8 contraction chunks for QKV

f32 = mybir.dt.float32
f32r = mybir.dt.float32r
bf16 = mybir.dt.bfloat16
DT = bf16  # matmul operand dtype

_COMPILED = None


def _build():
    nc = bacc.Bacc("TRN2", debug=False, num_devices=N_CORES)

    A = nc.dram_tensor("A", [4 * C, CQ], DT, kind="ExternalInput").ap()
    Wqkv = nc.dram_tensor("Wqkv", [C, 3 * OL], DT, kind="ExternalInput").ap()
    WpT = nc.dram_tensor("WpT", [OL, C], DT, kind="ExternalInput").ap()
    BQ = nc.dram_tensor("BQ", [OL, 1], f32, kind="ExternalInput").ap()
    Y = nc.dram_tensor("Y", [T, C], f32, kind="ExternalOutput").ap()

    Exp = mybir.ActivationFunctionType.Exp

    with tile.TileContext(nc) as tc:
        with tc.tile_pool(name="sbuf", bufs=1) as pool, \
             tc.tile_pool(name="work", bufs=1) as wpool, \
             tc.tile_pool(name="psum", bufs=1, space="PSUM") as psum:

            # ---- resident inputs (piece-contiguous loads, weights first) ----
            a_t, w_t = [], []
            for kc in range(KC):
                at = pool.tile([128, T], DT, tag=f"A{kc}", name=f"a{kc}")
                a_t.append(at)
                wt = pool.tile([128, 3 * OL], DT, tag=f"W{kc}", name=f"w{kc}")
                w_t.append(wt)
            # A is host-repacked so block (kc, piece) = rows
            # (kc*4+piece)*128..+128 is one contiguous 128KB read. Load
            # w[kc] + piece-0 of a[kc] interleaved so the first QKV
            # accumulation chain (needs all kc) completes earliest.
            for kc in range(KC):
                nc.sync.dma_start(w_t[kc][:], Wqkv[kc * 128:(kc + 1) * 128, :])
                blk = kc * 4 * 128
                nc.sync.dma_start(a_t[kc][:, 0:CQ], A[blk:blk + 128, 0:CQ])
            for piece in range(1, 4):
                for kc in range(KC):
                    blk = (kc * 4 + piece) * 128
                    nc.sync.dma_start(
                        a_t[kc][:, piece * CQ:(piece + 1) * CQ],
                        A[blk:blk + 128, 0:CQ])
            wp_t = []
            for kc in range(2):
                wp = pool.tile([128, C], DT, tag=f"WP{kc}", name=f"wp{kc}")
                nc.sync.dma_start(wp[:], WpT[kc * 128:(kc + 1) * 128, :])
                wp_t.append(wp)
            bq_t = []
            for m in range(2):
                bq = pool.tile([128, 1], f32, tag=f"BQ{m}", name=f"bq{m}")
                nc.sync.dma_start(bq[:], BQ[m * 128:(m + 1) * 128, :])
                bq_t.append(bq)
            col1 = pool.tile([128, 1], f32, tag="col1")
            nc.vector.memset(col1[:], 1.0)
            # warm the GpSimd ucode paths so the first real mask/broadcast
            # doesn't eat the cold-start cost mid-attention
            warm = wpool.tile([128, 8], f32, tag="warm")
            nc.vector.memset(warm[:], 1.0)
            nc.gpsimd.affine_select(
                out=warm[:], in_=warm[:],
                compare_op=mybir.AluOpType.is_ge, fill=0.0, base=0,
                pattern=[[1, 8]], channel_multiplier=-1)
            warm2 = wpool.tile([128, 8], f32, tag="warm2")
            nc.gpsimd.partition_broadcast(warm2[:], warm[0:1, :])

            # ---- persistent intermediates ----
            qt_sb = [pool.tile([128, T], DT, tag=f"QT{i}", name=f"qt{i}")
                     for i in range(2)]
            kt_sb = [pool.tile([128, T], DT, tag=f"KT{i}", name=f"kt{i}")
                     for i in range(2)]
            v_sb = [pool.tile([128, H_LOC * (D + 1)], DT, tag=f"V{i}",
                              name=f"v{i}") for i in range(NT)]
            ao_sb = [pool.tile([128, T], DT, tag=f"AO{i}", name=f"ao{i}")
                     for i in range(2)]

            # ---- phase 1a: Q^T, K^T (evict on DVE; ACT is Exp-only) ----
            for m in range(4):
                for n in range(T // CQ):
                    ps = psum.tile([128, CQ], f32, tag="prj", bufs=2, name="ps")
                    for kc in range(KC):
                        nc.tensor.matmul(
                            ps[:],
                            w_t[kc][:, m * 128:(m + 1) * 128],
                            a_t[kc][:, n * CQ:(n + 1) * CQ],
                            start=(kc == 0), stop=(kc == KC - 1))
                    if m < 2:
                        nc.vector.tensor_scalar_add(
                            qt_sb[m][:, n * CQ:(n + 1) * CQ], ps[:],
                            bq_t[m][:, 0:1])
                    else:
                        nc.vector.tensor_copy(
                            kt_sb[m - 2][:, n * CQ:(n + 1) * CQ], ps[:])

            # ---- phase 1b: V natural layout ----
            for tt in range(NT):
                ps = psum.tile([128, CQ], f32, tag="mm", bufs=4, name="psv")[:, 0:OL]
                for kc in range(KC):
                    nc.tensor.matmul(
                        ps[:],
                        a_t[kc][:, tt * 128:(tt + 1) * 128],
                        w_t[kc][:, 2 * OL:3 * OL],
                        start=(kc == 0), stop=(kc == KC - 1))
                for h in range(H_LOC):
                    nc.vector.tensor_copy(
                        v_sb[tt][:, h * (D + 1):h * (D + 1) + D],
                        ps[:, h * D:(h + 1) * D])
                    nc.vector.tensor_copy(
                        v_sb[tt][:, h * (D + 1) + D:(h + 1) * (D + 1)],
                        col1[:])

            # ---- phase 2+3: causal flash attention, proj interleaved ----
            # Per (h, qj): S chunks software-pipelined 2 ahead of PV so the
            # PE (in-order queue) never waits on the ACT exp; diagonal
            # chunks first so the GpSimd mask latency hides behind the
            # remaining S matmuls.
            for qj in range(T // CQ):          # 512-wide q chunk
                q0 = qj * CQ
                for h in range(H_LOC):
                    ht, hp = h // 2, (h % 2) * 64
                    n_kc = (qj + 1) * (CQ // CK)
                    order = list(range(qj * 4, n_kc)) + list(range(0, qj * 4))
                    pv = psum.tile([D + 1, CQ], f32, tag="pv", bufs=2)
                    pts = {}

                    def emit_s(kc, ht=ht, hp=hp, q0=q0, qj=qj, pts=pts):
                        sp = psum.tile([128, CQ], f32, tag="mm", bufs=4)
                        nc.tensor.matmul(
                            sp[:],
                            kt_sb[ht][hp:hp + D, kc * CK:(kc + 1) * CK],
                            qt_sb[ht][hp:hp + D, q0:q0 + CQ],
                            start=True, stop=True)
                        pt = wpool.tile([128, CQ], DT, tag="pT", bufs=8)
                        nc.scalar.activation(pt[:], sp[:], Exp, scale=1.0 / 8.0)
                        if kc >= qj * 4:   # diagonal chunk: mask q < k
                            nc.gpsimd.affine_select(
                                out=pt[:], in_=pt[:],
                                compare_op=mybir.AluOpType.is_ge,
                                fill=0.0, base=q0 - kc * CK,
                                pattern=[[1, CQ]], channel_multiplier=-1)
                        pts[kc] = pt

                    for j in range(min(3, n_kc)):
                        emit_s(order[j])
                    for i, kc in enumerate(order):
                        if i + 3 < n_kc:
                            emit_s(order[i + 3])
                        nc.tensor.matmul(
                            pv[:],
                            v_sb[kc][:, h * (D + 1):(h + 1) * (D + 1)],
                            pts.pop(kc),
                            start=(i == 0), stop=(i == n_kc - 1))
                    # normalize: evacuate PV to SBUF first (releases the
                    # PSUM slot ~2us earlier for the next block), then
                    # approx-recip the ones-row, broadcast, multiply.
                    pvs = wpool.tile([D, CQ], f32, tag="pvs", bufs=3)
                    nc.vector.tensor_copy(pvs[:], pv[0:D, :])
                    ls = wpool.tile([1, CQ], f32, tag="ls", bufs=2)
                    nc.vector.tensor_copy(ls[:], pv[D:D + 1, :])
                    r = wpool.tile([1, CQ], f32, tag="r", bufs=2)
                    with nc.allow_low_precision(reason="softmax denom"):
                        # approx_fast needs SBUF input at partition base 0
                        nc.vector.reciprocal_approx_fast(r[:], ls[:])
                    rbs = wpool.tile([D, CQ], f32, tag="rbs", bufs=2)
                    nc.gpsimd.partition_broadcast(rbs[:], r[:])
                    nc.vector.tensor_mul(
                        ao_sb[ht][hp:hp + D, q0:q0 + CQ],
                        pvs[:], rbs[:])

                # proj + store for this chunk's token tiles (overlaps the
                # next chunk's attention)
                for tt in range(qj * (CQ // 128), (qj + 1) * (CQ // 128)):
                    for n in range(2):
                        ps = psum.tile([128, CQ], f32, tag="prj", bufs=2,
                                       name="psp")
                        for kc in range(2):
                            nc.tensor.matmul(
                                ps[:],
                                ao_sb[kc][:, tt * 128:(tt + 1) * 128],
                                wp_t[kc][:, n * CQ:(n + 1) * CQ],
                                start=(kc == 0), stop=(kc == 1))
                        yt = wpool.tile([128, CQ], f32, tag="y", bufs=3)
                        nc.vector.tensor_copy(yt[:], ps[:])
                        nc.sync.dma_start(
                            Y[tt * 128:(tt + 1) * 128, n * CQ:(n + 1) * CQ],
                            yt[:])

    nc.compile()
    return nc


def _get_compiled():
    global _COMPILED
    if _COMPILED is None:
        _COMPILED = _build()
    return _COMPILED


def make_in_maps(x, Wq, bq, Wk, Wv, Wp):
    in_maps = []
    for c in range(N_CORES):
        b, g = divmod(c, 4)
        sl = slice(g * OL, (g + 1) * OL)
        in_maps.append({
            "A": np.ascontiguousarray(
                x[b].T.reshape(KC, 128, 4, CQ).transpose(0, 2, 1, 3)
                .reshape(4 * C, CQ)).astype(NP_DT),
            "Wqkv": np.concatenate(
                [Wq[sl].T, Wk[sl].T, Wv[sl].T], axis=1).astype(NP_DT),
            "WpT": np.ascontiguousarray(Wp[:, sl].T).astype(NP_DT),
            "BQ": bq[sl].reshape(OL, 1).astype(np.float32),
        })
    return in_maps


def kernel(x, Wq, bq, Wk, bk, Wv, bv, Wp, bp):
    x = np.asarray(x, dtype=np.float32)
    Wq = np.asarray(Wq, dtype=np.float32)
    bq = np.asarray(bq, dtype=np.float32)
    Wk = np.asarray(Wk, dtype=np.float32)
    Wv = np.asarray(Wv, dtype=np.float32)
    Wp = np.asarray(Wp, dtype=np.float32)
    bv = np.asarray(bv, dtype=np.float32)
    bp = np.asarray(bp, dtype=np.float32)

    nc = _get_compiled()

    in_maps = make_in_maps(x, Wq, bq, Wk, Wv, Wp)

    res = run_bass_kernel_spmd(nc, in_maps, core_ids=list(range(N_CORES)))

    extra = bv @ Wp.T + bp  # bv/bp fold out of the device kernel
    out = np.empty((B, T, C), dtype=np.float32)
    for b in range(B):
        acc = res.results[4 * b]["Y"].astype(np.float32)
        for g in range(1, 4):
            acc = acc + res.results[4 * b + g]["Y"]
        out[b] = acc + extra
    return out
